# revision 70
# baseline (speedup 1.0000x reference)
"""Multi-head attention (B=4, S=2048, D=2048, H=16, dk=128) on 8 TRN2 NeuronCores.

Sharding: core c handles batch b = c // 2 and the 8 heads hh = c % 2
(heads hh*8 .. hh*8+8) over the FULL sequence.  No redundant K/V projections.
After attention, the two cores of a batch pair exchange attention outputs for
the query rows they don't own via a pairwise AllGather collective, then each
core runs the output projection for its own 1024 query rows.

v12 scheduling (from v3, after perfetto/NTFF analysis; kernel is PE-bound at
~3.6k matmul passes x 512 cols, p50 216ns/pass when the clock is cool):
  - the NEXT head's K/Q-projection passes interleave INTO the attention
    t-loop (2 per score tile, via generator) so the 5-pass t-cadence always
    covers the scalar engine's ~0.7us exp latency;
  - V-phase DMA is need-ordered around the all-cores HBM contention wall
    (~140GB/s early): first halves of x^T + w_v[0] first, then w_v[1], then
    second halves, with the matmul group sequence consuming h1-only column
    groups for BOTH w_v chunks before any h2 group;
  - peer-half attention outputs land in separate aop tiles (whole-tile dep
    granularity would delay the exchange send by ~20us), which then double
    as the gather receive buffers;
  - the final chunk's PSUM accumulators come from the idle K/Q-proj banks so
    the out-projection pool's banks are drained early;
  - the out-projection defers the j14/j15 (peer heads 6/7) closing passes of
    all four oc=0 chain-pairs (~26us of cover) until the last head's
    exchange lands, and pairs chains 2-at-a-time across PSUM banks.

SPMD uniformity: the program is identical on all cores.  Divergence lives in
host-prepared data only:
  - x^T arrives with the core's OWNED query columns rolled to the front, so
    "my rows" are always local columns 0..1023 and the slice sent to the peer
    is always local columns 1024..2047.  (The rolled orders of the two pair
    members are mutually inverse, so the sent slice lands in exactly the
    receiver's local row order.)
  - the AllGather output has slot s = pair-member s's contribution; which slot
    holds the PEER's heads depends on parity, so the readback is a dma_gather
    whose 128 row indices (peer_slot*128 + r) are a host-provided per-core
    input — the out-projection then runs exactly 16 accumulating matmuls per
    PSUM tile (8 own heads from SBUF + 8 gathered peer heads).

All matmul operands bfloat16 (PSUM fp32): bf16 halves the PE's self-loading
LDWEIGHTS time (~112ns/tile vs fp32r's ~224ns), which hides fully under N=512
matmul compute (213ns) -> ~1 col/cycle.

Layout (zero on-chip transposes) as before: Q^T/K^T transposed via w-col-block
lhsT; V natural; scores transposed [sk, sq]; P^T = exp(scores^T/sqrt(dk)) with
no max subtraction; row sums via ones-matmul; attn_out^T = V_h^T @ P^T with
1/rowsum folded in via a PE-broadcast reciprocal.
"""

import os
import sys

import numpy as np

for _p in ("/opt/trn_rl_repo", "/root/.axon_site/_ro/trn_rl_repo"):
    if os.path.isdir(_p) and _p not in sys.path:
        sys.path.insert(0, _p)

P = 128

_CACHE = {}

PAIRS = [[0, 1], [2, 3], [4, 5], [6, 7]]


def _bf16(a):
    import ml_dtypes

    return np.ascontiguousarray(a, dtype=np.float32).astype(ml_dtypes.bfloat16)


def build_nc(D=2048, S=2048):
    """Build the single-core Bass program (SPMD: identical on all cores)."""
    from contextlib import ExitStack

    import concourse.tile as tile
    from concourse import bacc, mybir

    F32 = mybir.dt.float32
    BF16 = mybir.dt.bfloat16
    Exp = mybir.ActivationFunctionType.Exp

    HQ = 8              # local heads per core
    DT = D // P         # d-model tiles (16)
    ST = S // P         # key tiles (16)
    SQ = S              # attention runs over ALL queries
    SQO = S // 2        # owned query rows (out-projection)
    NSKC = S // 512     # K^T / Q^T projection free-dim chunks (4)
    VC = 512            # w_v streaming chunk width
    NVC = (HQ * P) // VC    # 2 chunks cover this core's 1024 V columns
    OC = 512            # w_o streaming chunk width
    NOC = D // OC
    NWO = HQ + HQ       # 16 lhsT slots in the out-projection (own + peer)
    scale = float(1.0 / np.sqrt(128.0))

    nc = bacc.Bacc("TRN2", target_bir_lowering=False, debug=False,
                   num_devices=8)

    xt_d = nc.dram_tensor("xt", [D, S], BF16, kind="ExternalInput").ap()
    ones_d = nc.dram_tensor("ones", [P, P], BF16, kind="ExternalInput").ap()
    wq_d = nc.dram_tensor("wq", [HQ, D, P], BF16, kind="ExternalInput").ap()
    wk_d = nc.dram_tensor("wk", [HQ, D, P], BF16, kind="ExternalInput").ap()
    wv_d = nc.dram_tensor("wv", [NVC, D, VC], BF16, kind="ExternalInput").ap()
    wo_d = nc.dram_tensor("wo", [NOC, NWO * P, OC], BF16,
                          kind="ExternalInput").ap()
    out_d = nc.dram_tensor("out", [SQO, D], F32, kind="ExternalOutput").ap()

    mm = nc.tensor.matmul

    # per-core gather indices: select the PEER's slot out of the AllGather
    # output (host supplies 1 - parity, so the program stays SPMD-uniform)
    gidx_d = nc.dram_tensor("gidx", [P, 8], mybir.dt.int16,
                            kind="ExternalInput").ap()

    with tile.TileContext(nc) as tc, \
            nc.allow_low_precision(reason="bf16 matmul operands"):
        with ExitStack() as octx:
            const = octx.enter_context(tc.tile_pool(name="const", bufs=1))
            ones_sb = const.tile([P, P], BF16)
            gidx_sb = const.tile([P, 8], mybir.dt.int16)

            # DRAM bounce buffers for the pairwise attention-output exchange.
            # Head 7's exchange is split into two 512-col halves so the
            # second collective finishes before the out-projection needs
            # agr[7] (a whole-head exchange lands ~15us too late).
            dram = octx.enter_context(
                tc.tile_pool(name="agd", bufs=1, space="DRAM"))
            ag_in = [dram.tile([P, SQO], BF16, name=f"agi{h}")
                     for h in range(HQ)]
            ag_out = [dram.tile([2, P, SQO], BF16, name=f"ago{h}")
                      for h in range(HQ)]

            # Peer-half attention outputs live in SEPARATE tiles (aop_sb):
            # cross-engine deps are whole-tile, so the exchange DMA would
            # otherwise wait for the own-half chunks too (~20us later).
            # After the send DMA copies aop to DRAM, the SAME tile receives
            # the gathered peer slot (the gpsimd queue orders send ->
            # collective -> gather, so the overwrite is safe) — no separate
            # 16KB receive pool.
            ao_pool = octx.enter_context(tc.tile_pool(name="aop", bufs=HQ))
            aop_pool = octx.enter_context(tc.tile_pool(name="aopp", bufs=HQ))
            # w_o chunk-0 OWN-heads prefetch (single 8KB tile; the peer half
            # loads from wo3 at out-proj start) — loaded during head-7
            # attention so the out-projection starts without a DMA stall
            wop = octx.enter_context(tc.tile_pool(name="wop", bufs=1))
            ao_sb = [ao_pool.tile([P, SQO], BF16, name=f"ao{h}", tag="ao")
                     for h in range(HQ)]
            aop_sb = [aop_pool.tile([P, 1, SQO], BF16, name=f"aop{h}",
                                    tag="aop")
                      for h in range(HQ)]

            # persistent SBUF residents; xt/v die before the out-projection
            # (opened LAST so releasing them keeps pool stack order).
            mainctx = octx.enter_context(ExitStack())
            xt_pool = mainctx.enter_context(tc.tile_pool(name="xtp", bufs=DT))
            v_pool = mainctx.enter_context(tc.tile_pool(name="vp", bufs=ST))

            xt_sb = [xt_pool.tile([P, S], BF16, name=f"xts{dt}", tag="xt")
                     for dt in range(DT)]
            v_sb = [v_pool.tile([P, HQ * P], BF16, name=f"vs{t}", tag="v")
                    for t in range(ST)]

            # K/Q-projection pools outlive the V phase (their 2 PSUM banks +
            # the V phase's 6 fit together), so head 0's projection chains
            # interleave with the tail of the V phase on the PE.
            fctx = mainctx.enter_context(ExitStack())
            wqk = fctx.enter_context(tc.tile_pool(name="wqk", bufs=1))
            iok = fctx.enter_context(tc.tile_pool(name="iok", bufs=1))
            ps_kq = fctx.enter_context(
                tc.tile_pool(name="pskq", bufs=2, space="PSUM"))

            k2s, q2s = {}, {}

            def proj_steps(h, pspool=None, pbufs=2):
                """Generator: one yield per PE pass of head h's K/Q proj.

                Yielding lets the attention t-loop interleave 2 projection
                passes per score tile, so the PE's t-cadence (5 passes) always
                covers the scalar engine's 0.7-0.85us exp latency.
                """
                pool = pspool if pspool is not None else ps_kq
                for w_d, out_tag, store in ((wk_d, "k2", k2s),
                                            (wq_d, "q2", q2s)):
                    wb = wqk.tile([P, DT, P], BF16, name="wb", tag="w", bufs=2)
                    nc.sync.dma_start(
                        out=wb[:], in_=w_d[h].rearrange("(t p) n -> p t n", p=P))
                    o2 = iok.tile([P, S], BF16, name=out_tag, tag=out_tag,
                                  bufs=2)
                    store[h] = o2
                    for g in range(NSKC // 2):
                        ps = [pool.tile([P, 512], F32, name=f"ps{c}",
                                        tag="ps", bufs=pbufs)
                              for c in range(2)]
                        for dt in range(DT):
                            for c in range(2):
                                sk = (2 * g + c) * 512
                                mm(ps[c][:], wb[:, dt, :],
                                   xt_sb[dt][:, sk:sk + 512],
                                   start=(dt == 0), stop=(dt == DT - 1))
                                yield
                        for c in range(2):
                            sk = (2 * g + c) * 512
                            nc.vector.tensor_copy(o2[:, sk:sk + 512], ps[c][:])

            def emit_kq_proj(h, **kw):
                for _ in proj_steps(h, **kw):
                    pass

            # ---------------- Phase V: V = x @ w_v (natural layout) ----------------
            # dt-OUTER loop order so the first matmuls only need xt tile 0 +
            # the first w_v chunk -> PE starts while x^T is still streaming in.
            with ExitStack() as ctx:
                wvp = ctx.enter_context(tc.tile_pool(name="wvp", bufs=1))
                psV = ctx.enter_context(
                    tc.tile_pool(name="psV", bufs=1, space="PSUM"))

                # DMA plan, need-ordered (all-8-cores HBM contention caps the
                # early stream at ~140GB/s aggregate, so delivery order must
                # match consumption): first halves (cols 0:1024) of all xt
                # tiles + w_v[0] slices, then w_v[1], then second halves.
                # The matmul group sequence below consumes h1-only column
                # groups for BOTH jb chunks first (256 passes, ~55us) —
                # by then w_v[1] and the second halves have landed.
                # separate buffers (bufs=2): the group sequence interleaves
                # jb=1 groups before jb=0's h2 groups, so both live at once
                wvb = [wvp.tile([P, DT, VC], BF16, name=f"wvb{jb}", tag="wv",
                                bufs=2)
                       for jb in range(NVC)]
                ap0 = wv_d[0].rearrange("(t p) n -> p t n", p=P)
                ap1 = wv_d[1].rearrange("(t p) n -> p t n", p=P)
                HS = S // 2
                # wv[0:2] leads the scalar queue so the first matmul's two
                # gates (xt0-h1 on sync, wv slice 0 on scalar) load in
                # parallel instead of back-to-back on sync
                nc.scalar.dma_start(out=wvb[0][:, 0:2, :], in_=ap0[:, 0:2, :])
                nc.scalar.dma_start(out=ones_sb[:], in_=ones_d[:])
                nc.scalar.dma_start(out=gidx_sb[:], in_=gidx_d[:])
                nc.sync.dma_start(out=xt_sb[0][:, 0:HS], in_=xt_d[0:P, 0:HS])
                for dt in range(1, DT):
                    eng = nc.sync if dt % 2 == 0 else nc.scalar
                    eng.dma_start(out=xt_sb[dt][:, 0:HS],
                                  in_=xt_d[dt * P:(dt + 1) * P, 0:HS])
                    if 1 < dt < 9:
                        lo, hi = 2 * (dt - 1), 2 * dt
                        weng = nc.scalar if dt % 2 == 0 else nc.sync
                        weng.dma_start(out=wvb[0][:, lo:hi, :],
                                       in_=ap0[:, lo:hi, :])
                nc.sync.dma_start(out=wvb[1][:, 0:8, :], in_=ap1[:, 0:8, :])
                nc.scalar.dma_start(out=wvb[1][:, 8:DT, :],
                                    in_=ap1[:, 8:DT, :])
                for dt in range(DT):
                    eng = nc.sync if dt % 2 == 0 else nc.scalar
                    eng.dma_start(out=xt_sb[dt][:, HS:S],
                                  in_=xt_d[dt * P:(dt + 1) * P, HS:S])

                # group sequence: h1-only column groups for both jb first
                for jb, (lo, hi) in ((0, (0, 6)), (0, (6, 8)), (1, (0, 6)),
                                     (1, (6, 8)), (0, (8, 11)), (0, (11, 14)),
                                     (0, (14, 16)), (1, (8, 11)),
                                     (1, (11, 14)), (1, (14, 16))):
                    psv = [psV.tile([P, VC], F32, name=f"psv{k}",
                                    tag=f"psv{k % 3}", bufs=2)
                           for k in range(hi - lo)]
                    for dt in range(DT):
                        for k in range(hi - lo):
                            kt = lo + k
                            mm(psv[k][:], xt_sb[dt][:, kt * P:(kt + 1) * P],
                               wvb[jb][:, dt, :],
                               start=(dt == 0), stop=(dt == DT - 1))
                    for k in range(hi - lo):
                        kt = lo + k
                        nc.vector.tensor_copy(
                            v_sb[kt][:, jb * VC:(jb + 1) * VC], psv[k][:])

            # head 0's K/Q projection interleaves with the V-phase tail.
            # It runs un-interleaved (no attention yet), so give it 4 PSUM
            # banks (free until the attention pools open) — with 2, each
            # 32-pass group stalls ~1us on the previous group's DVE drain.
            with tc.tile_pool(name="pskq0", bufs=1, space="PSUM") as ps_kq0:
                emit_kq_proj(0, pspool=ps_kq0, pbufs=4)

            # ------ Fused: per-head K/Q projection (SBUF-resident) + attention ------
            with ExitStack() as ctx:
                pt_pool = ctx.enter_context(tc.tile_pool(name="ptp", bufs=1))
                sm2 = ctx.enter_context(tc.tile_pool(name="sm2", bufs=1))
                ps_pt = ctx.enter_context(
                    tc.tile_pool(name="pspt", bufs=2, space="PSUM"))
                ps_ov = ctx.enter_context(
                    tc.tile_pool(name="psov", bufs=2, space="PSUM"))
                ps_sm = ctx.enter_context(
                    tc.tile_pool(name="pssm", bufs=2, space="PSUM"))

                # LEAD=3: the chunk-tail flush is 6 back-to-back psb/pso
                # matmuls (1.28us), which covers the last exp's ~0.7us
                # latency — LEAD=2 left a ~0.5-1us PE bubble per chunk.
                LEAD = 3

                def emit_attention(h, steps=None):
                    # peer-half chunks (2, 3) first so the exchange for this
                    # head can overlap the own-half chunks (0, 1).
                    # `steps` is the NEXT head's K/Q-projection pass generator:
                    # 2 projection passes per score tile (128 over the head)
                    # keep the PE t-cadence at 5 passes, so the scalar
                    # engine's exp latency never gates the psb/pso chains.
                    k2, q2 = k2s[h], q2s[h]
                    # head 7 has no interleaved proj passes: its t-cadence is
                    # 3 passes (648ns) < exp latency (686ns), so the scalar
                    # queue drifts ~0.7us behind per chunk — LEAD=8 makes the
                    # 16-pass flush (4.1us) cover the accumulated drift.
                    lead = LEAD if steps is not None else 8
                    for sqc in (2, 3, 0, 1):
                        if h == HQ - 1 and sqc == 1:
                            # the kernel's FINAL chunk: take the idle K/Q-proj
                            # banks so ps_ov/ps_sm are already drained when
                            # the out-projection pool claims its 6 banks —
                            # otherwise its first matmul waits ~4.4us for
                            # this chunk's reciprocal+mul on the DVE
                            pso = ps_kq.tile([P, 512], F32, name="pso7",
                                             tag="ps", bufs=2)
                            psb = ps_kq.tile([P, 512], F32, name="psb7",
                                             tag="ps", bufs=2)
                        else:
                            pso = ps_ov.tile([P, 512], F32, name="pso")
                            psb = ps_sm.tile([P, 512], F32, name="psb")
                        ptts = [None] * ST
                        for t in range(ST + lead):
                            if t < ST:
                                pst = ps_pt.tile([P, 512], F32, name="pst")
                                mm(pst[:], k2[:, t * P:(t + 1) * P],
                                   q2[:, sqc * 512:(sqc + 1) * 512],
                                   start=True, stop=True)
                                ptt = pt_pool.tile([P, 512], BF16, name="ptt",
                                                   tag="pt", bufs=9)
                                nc.scalar.activation(ptt[:], pst[:], Exp,
                                                     scale=scale)
                                ptts[t] = ptt
                            if t >= lead:
                                u = t - lead
                                mm(psb[:], ones_sb[:], ptts[u][:],
                                   start=(u == 0), stop=(u == ST - 1))
                                mm(pso[:], v_sb[u][:, h * P:(h + 1) * P],
                                   ptts[u][:],
                                   start=(u == 0), stop=(u == ST - 1))
                            if t < ST and steps is not None:
                                next(steps, None)
                                next(steps, None)
                        rbc = sm2.tile([P, 512], F32, name="rbc", tag="rbc",
                                       bufs=2)
                        nc.vector.reciprocal(rbc[:], psb[:])
                        if sqc >= 2:
                            dst = aop_sb[h][:, 0,
                                            (sqc - 2) * 512:(sqc - 1) * 512]
                        else:
                            dst = ao_sb[h][:, sqc * 512:(sqc + 1) * 512]
                        nc.vector.tensor_mul(dst, pso[:], rbc[:])
                        if sqc == 3:
                            emit_exchange(h)

                def emit_exchange(h):
                    # send my aoT for the PEER's rows (local cols SQO..), pair-
                    # AllGather, then gather ONLY the peer's slot back using
                    # the host-provided per-core index rows.  The send DMA
                    # rides the sync queue (the gpsimd queue's conservative
                    # semaphores fire it ~2 chunks late); the collective and
                    # gather stay on gpsimd, ordered by data deps.
                    nc.sync.dma_start(out=ag_in[h][:], in_=aop_sb[h][:, 0, :])
                    nc.gpsimd.collective_compute(
                        "AllGather",
                        mybir.AluOpType.bypass,
                        replica_groups=PAIRS,
                        ins=[ag_in[h][:]],
                        outs=[ag_out[h][:]],
                    )
                    nc.gpsimd.dma_gather(
                        out_ap=aop_sb[h][:],
                        in_ap=ag_out[h].rearrange("s p n -> (s p) n"),
                        idxs_ap=gidx_sb[:],
                        num_idxs=P,
                        num_idxs_reg=P,
                        elem_size=SQO,
                    )

                wo_pre = None
                for h in range(HQ):
                    steps = proj_steps(h + 1) if h + 1 < HQ else None
                    if h == HQ - 2:
                        # prefetch w_o[0]'s own-head half during the tail,
                        # plus its j14/j15 slots for the final peer sweep
                        wo_pre = wop.tile([P, HQ, OC], BF16, name="wopre",
                                          tag="wopre")
                        wo_pr2 = wop.tile([P, HQ, OC], BF16, name="wopr2",
                                          tag="wopr2")
                        wo_tl = wop.tile([P, 2, OC], BF16, name="wotl",
                                         tag="wotl")
                        ap_wo0 = wo_d[0].rearrange("(t p) n -> p t n", p=P)
                        nc.scalar.dma_start(out=wo_pre[:],
                                            in_=ap_wo0[:, 0:HQ, :])
                        nc.scalar.dma_start(out=wo_pr2[:],
                                            in_=ap_wo0[:, HQ:NWO, :])
                        nc.scalar.dma_start(out=wo_tl[:],
                                            in_=ap_wo0[:, NWO - 2:NWO, :])
                    emit_attention(h, steps)
                    if steps is not None:
                        for _ in steps:  # exhaust any leftover proj passes
                            pass

            # close xt/v pools before the out-projection scope
            mainctx.close()

            # ---------------- Out-projection (own 1024 rows) ----------------
            # 16 lhsT slots per PSUM tile: 8 own heads (SBUF) + 8 received
            # peer heads.  The host orders w_o rows [own heads | peer heads].
            with ExitStack() as ctx:
                wo3 = ctx.enter_context(tc.tile_pool(name="wo3", bufs=1))
                ev3 = ctx.enter_context(tc.tile_pool(name="ev3", bufs=1))
                # bufs=6, not 8: with 8 the pool would also inherit the two
                # banks of the kernel's final attention chunk, whose release
                # waits on its reciprocal tail on the DVE (~4us PE stall at
                # the phase transition)
                ps3p = ctx.enter_context(
                    tc.tile_pool(name="ps3p", bufs=1, space="PSUM"))
                # oc=0's j0..13 partial sums park here until the final sweep
                o0p = ctx.enter_context(tc.tile_pool(name="o0p", bufs=8))
                o0ev = [o0p.tile([P, OC], F32, name=f"o0ev{t}", tag="o0ev")
                        for t in range(SQO // P)]

                # chains run in PAIRS (two PSUM banks, j-interleaved): the
                # next matmul always targets the other bank, so its weight
                # load pipelines behind the current accumulation
                for oc in range(NOC):
                    if oc == 0:
                        # both halves of w_o[0] were prefetched into wop —
                        # wo3's SBUF region reuses the freed xt/v space, so
                        # any oc=0 load there would gate on the attention
                        # drain (~4us stall at the j8 passes)
                        rhs_of = lambda j: (wo_pre[:, j, :] if j < HQ
                                            else wo_pr2[:, j - HQ, :])
                    else:
                        wob = wo3.tile([P, NWO, OC], BF16, name="wob",
                                       tag="wo", bufs=2)
                        nc.sync.dma_start(
                            out=wob[:],
                            in_=wo_d[oc].rearrange("(t p) n -> p t n", p=P))
                        rhs_of = lambda j, wob=wob: wob[:, j, :]
                    def lhsT_of(j, sl):
                        if j < HQ:
                            return ao_sb[j][:, sl]
                        return aop_sb[j - HQ][:, 0, sl]

                    def close_pair(ps3, sls, js):
                        for j in js:
                            for i in range(2):
                                mm(ps3[i][:], lhsT_of(j, sls[i]), rhs_of(j),
                                   start=False, stop=(j == NWO - 1))
                        for i in range(2):
                            oev = ev3.tile([P, OC], F32, name="oev",
                                           tag="oev", bufs=6)
                            nc.vector.tensor_copy(oev[:], ps3[i][:])
                            # scalar queue: the sync queue carries the 2MB
                            # wob loads — don't serialize outputs behind them
                            nc.scalar.dma_start(
                                out=out_d[sls[i], oc * OC:(oc + 1) * OC],
                                in_=oev[:])

                    # oc=0 chains stop at j13: their j14/j15 (peer heads 6/7)
                    # contributions move to the final sweep below, so the
                    # LAST pair exchange isn't needed until the kernel's
                    # last ~10us — robust to cross-core clock skew, which
                    # can delay the head-7 AllGather by 50us+
                    for sp in range(SQO // P // 2):
                        ps3 = [ps3p.tile([P, OC], F32, name=f"ps3{i}",
                                         tag="ps3", bufs=6)
                               for i in range(2)]
                        sls = [slice((2 * sp + i) * P, (2 * sp + i + 1) * P)
                               for i in range(2)]
                        nacc = NWO - 2 if oc == 0 else NWO - 1
                        for j in range(nacc):
                            for i in range(2):
                                mm(ps3[i][:], lhsT_of(j, sls[i]), rhs_of(j),
                                   start=(j == 0),
                                   stop=(oc == 0 and j == nacc - 1))
                        if oc == 0:
                            for i in range(2):
                                nc.vector.tensor_copy(
                                    o0ev[2 * sp + i][:], ps3[i][:])
                        else:
                            close_pair(ps3, sls, js=(NWO - 1,))

                    if oc == 1:
                        # peer sweep: oc=0's j14/j15 (peer heads 6/7)
                        # partials merge with the parked j0..13 sums on the
                        # DVE, then write out.  Placed after oc=1 — still
                        # ~28us of cover for a skew-delayed last exchange,
                        # without serializing the sweep at the kernel tail.
                        for sqt in range(SQO // P):
                            psw = ps3p.tile([P, OC], F32, name="psw",
                                            tag="ps3", bufs=6)
                            sl = slice(sqt * P, (sqt + 1) * P)
                            for jj in range(2):
                                mm(psw[:], lhsT_of(NWO - 2 + jj, sl),
                                   wo_tl[:, jj, :],
                                   start=(jj == 0), stop=(jj == 1))
                            oev = ev3.tile([P, OC], F32, name="oev",
                                           tag="oev", bufs=6)
                            nc.vector.tensor_add(oev[:], o0ev[sqt][:],
                                                 psw[:])
                            nc.scalar.dma_start(out=out_d[sl, 0:OC],
                                                in_=oev[:])

    nc.compile()
    return nc


def prep_inputs(x, w_q, w_k, w_v, w_o, D=2048, S=2048, n_cores=8):
    """Host-side shard + re-layout. Returns in_maps for run_bass_kernel_spmd."""
    HQ = 8
    SQO = S // 2
    NOC = D // 512
    ones = _bf16(np.ones((P, P), dtype=np.float32))
    # per head-half: w_q/w_k column blocks, w_v column slice
    wq_h, wk_h, wv_h, wo_h = [], [], [], []
    for hh in range(2):
        hsl = slice(hh * HQ * P, (hh + 1) * HQ * P)
        wq_h.append(_bf16(w_q[:, hsl].reshape(D, HQ, P).transpose(1, 0, 2)))
        wk_h.append(_bf16(w_k[:, hsl].reshape(D, HQ, P).transpose(1, 0, 2)))
        wv_h.append(_bf16(w_v[:, hsl].reshape(D, 2, 512).transpose(1, 0, 2)))
        # out-projection slots: [own 8 heads' w_o rows | peer 8 heads' rows]
        psl = slice((1 - hh) * HQ * P, (2 - hh) * HQ * P)
        wo_pad = np.concatenate([w_o[hsl, :], w_o[psl, :]], axis=0)  # [16*P, D]
        wo_h.append(_bf16(
            wo_pad.reshape(16 * P, NOC, 512).transpose(1, 0, 2)))
    # gather indices: row k of the peer's AllGather slot, wrapped so that
    # idx position k lives at [k % 16, k // 16] (replicated to 128 partitions)
    gidx_h = []
    for hh in range(2):
        k = (np.arange(8)[None, :] * 16 + np.arange(128)[:, None] % 16)
        gidx_h.append(((1 - hh) * P + k).astype(np.int16))
    in_maps = []
    for c in range(n_cores):
        b, hh = divmod(c, 2)
        xt = x[b].T  # [D, S]
        # roll this core's OWNED query columns to the front
        xt = _bf16(np.roll(xt, -hh * SQO, axis=1))
        in_maps.append({
            "xt": xt, "wq": wq_h[hh], "wk": wk_h[hh], "wv": wv_h[hh],
            "wo": wo_h[hh], "ones": ones, "gidx": gidx_h[hh],
        })
    return in_maps


def run(x, w_q, w_k, w_v, w_o, trace=False):
    from concourse.bass_utils import run_bass_kernel_spmd

    B, S, D = x.shape
    n_cores = 8
    SQO = S // 2
    key = (D, S)
    if key not in _CACHE:
        _CACHE[key] = build_nc(D=D, S=S)
    nc = _CACHE[key]
    in_maps = prep_inputs(x, w_q, w_k, w_v, w_o, D=D, S=S, n_cores=n_cores)
    res = run_bass_kernel_spmd(nc, in_maps, core_ids=list(range(n_cores)), trace=trace)
    out = np.empty((B, S, D), dtype=np.float32)
    for c in range(n_cores):
        b, hh = divmod(c, 2)
        out[b, hh * SQO:(hh + 1) * SQO, :] = res.results[c]["out"]
    return out, res


def kernel(x, w_q, w_k, w_v, w_o):
    out, _ = run(np.asarray(x), np.asarray(w_q), np.asarray(w_k),
                 np.asarray(w_v), np.asarray(w_o))
    return out

